# revision 3
# baseline (speedup 1.0000x reference)
"""Distributed Trainium2 kernel for nn_Attn_77970836292156.

Cross-attention block: fused QKV projection + per-head RMSNorm + RoPE +
bf16 SDPA (4096 keys = 2048 self + 2048 cross) + output projection.

Sharding: tensor-parallel on heads. 16 heads / 8 cores = 2 heads per core.
W_qkv / W_ckv column-sharded by head; every core holds full x, y (transposed,
bf16). Attention runs fully local per core in a transposed layout
(head-dims on partitions, positions on the free axis). An AllToAll converts
head-sharding -> sequence-sharding, then each core applies the full W_out to
its position slice (row-sharded matmul accumulated over all 1024 dims).

Structure (v2 -- rebuilt from trace analysis of the previous baseline):
- Attention runs in FOUR 512-query chunks, each with its own AllToAll, so
  collectives pipeline under later chunks' compute and only the last
  chunk's (128KB) A2A is exposed, instead of one big A2A at the end.
- PSUM: pv accumulators [128,512] x2 heads double-buffered (4 banks) +
  QK score tiles (2 banks) + out-proj accumulators (2 banks) = 8 banks,
  so consecutive chunks never stall on each other's normalize.
- Every matmul has a 128-column stationary operand => fast-weight-load
  stays enabled (the old 65-wide PV stationary disabled FWL and cost
  ~100ns/matmul). The PV stationary per head is [v(64) | ones(64)] (or
  mirrored), so PSUM rows opposite the o-rows hold 64 copies of the
  softmax denominator: the reciprocal (ACT Ln + Exp(-x), same table set
  as the softmax Exp => no table reloads) is computed on 64 partitions
  directly and only needs a partition-shift DMA, no broadcast.
- Softmax exp split across engines per head: h0 on ACT (table exp), h1 on
  DVE via a Schraudolph bit-trick (bits = trunc(score*a + b) as int16,
  reinterpreted bf16).
- RMSNorm rsqrt per projection (3x Ln+Exp over [2,2048], same ACT table)
  emitted as soon as that projection's mean-squares finish, so q/k rope
  starts during the y-side projections instead of after everything.
- RoPE split: chunks needed first (q0, k0, k1) run on the DVE right at
  phase-1 end; the rest (k2-k7, q1) run on the otherwise-idle GpSimd
  DURING attention chunk 0; q2/q3 run on DVE in the chunk boundaries.
  In-place writes + byte-granular tile deps let attention consume each
  kc chunk as soon as its rope lands.
- A2A staging is ONE transposed-AP DMA per chunk (not 8), out-proj input
  one DMA per chunk; both out-projections are emitted after chunk 3 so
  pair 0's matmuls execute inside the final A2A's flight window.
- x/y are consumed through a 3-deep chunk pool: the first projection
  starts after one 1MB chunk load instead of the full 8MB input load.
"""

import os

import numpy as np
import ml_dtypes

import concourse.bass as bass
import concourse.tile as tile
from concourse import bacc, mybir
from concourse.bass_utils import run_bass_kernel_spmd

BF16 = mybir.dt.bfloat16
F32 = mybir.dt.float32
I16 = mybir.dt.int16
AF = mybir.ActivationFunctionType

# Problem constants (hardcoded per spec).
N = 2048        # query positions
M = 2048        # cross positions
NK = N + M      # total keys
D = 1024        # model dim
H = 16          # heads
DH = 64         # head dim
HL = 2          # heads per core
DL = HL * DH    # local head dims = 128
P = 128
NCORES = 8
EPS = 1e-6
ROPE_BASE = 10000.0
SCALE = 0.125   # 1/sqrt(64)
KC = NK // P    # 32 key chunks of 128
NCH = 4         # query chunks
CQ = N // NCH   # 512 queries per chunk
CPC = CQ // NCORES  # 64 positions per core per chunk

# Schraudolph exp constants for bf16 bits = trunc(score*EXA + EXB):
#   bits = 128*(score*SCALE*log2 e) + 127*128 - 5.5 (minimax centering)
#   + 0.5 (truncation compensation)
EXA = SCALE * 128.0 * 1.4426950408889634
EXB = 16251.0

LAST_RESULT = None  # test harness reads exec_time_ns from here


def build_nc():
    nc = bacc.Bacc()

    # ---------------- DRAM parameters ----------------
    # x/y arrive host-prearranged chunk-major [p, chunk, f, 512] so each
    # position-chunk load is one contiguous 8KB run per partition.
    xT = nc.declare_dram_parameter("xT", [P, 4, 8, 512], BF16, isOutput=False)
    yT = nc.declare_dram_parameter("yT", [P, 4, 8, 512], BF16, isOutput=False)
    wq = nc.declare_dram_parameter("wq", [P, 8, DL], BF16, isOutput=False)
    wk = nc.declare_dram_parameter("wk", [P, 8, DL], BF16, isOutput=False)
    wv = nc.declare_dram_parameter("wv", [P, 8, DL], BF16, isOutput=False)
    wck = nc.declare_dram_parameter("wck", [P, 8, DL], BF16, isOutput=False)
    wcv = nc.declare_dram_parameter("wcv", [P, 8, DL], BF16, isOutput=False)
    wo = nc.declare_dram_parameter("wo", [P, 8, D], BF16, isOutput=False)
    bo = nc.declare_dram_parameter("bo", [1, D], BF16, isOutput=False)
    cq = nc.declare_dram_parameter("cq", [P, N], BF16, isOutput=False)
    sq = nc.declare_dram_parameter("sq", [P, N], BF16, isOutput=False)
    ckc = nc.declare_dram_parameter("ckc", [P, NK], BF16, isOutput=False)
    cks = nc.declare_dram_parameter("cks", [P, NK], BF16, isOutput=False)
    hmask = nc.declare_dram_parameter("hmask", [P, HL], BF16, isOutput=False)
    hsel = nc.declare_dram_parameter("hsel", [HL, P], BF16, isOutput=False)
    ident = nc.declare_dram_parameter("ident", [P, P], BF16, isOutput=False)
    out_ext = nc.declare_dram_parameter("out", [2 * P, D], F32, isOutput=True)

    # A2A bounce buffers, one pair per q chunk (collectives can't touch I/O
    # tensors; separate tensors keep chunk deps independent).
    a2a_in = [nc.dram_tensor(f"a2a_in{c}", [NCORES, P, CPC], BF16)
              for c in range(NCH)]
    a2a_out = [nc.dram_tensor(f"a2a_out{c}", [NCORES, P, CPC], BF16)
               for c in range(NCH)]

    with tile.TileContext(nc) as tc, \
            tc.tile_pool(name="singles", bufs=1) as singles, \
            tc.tile_pool(name="bgrope", bufs=2) as bgrope:

        # ---------------- static SBUF loads ----------------
        def load_w(param):
            t = singles.tile([P, 8, DL], BF16, tag=param.name + "_sb")
            nc.sync.dma_start(out=t, in_=param[:, :, :])
            return t

        wq_sb = load_w(wq)
        wk_sb = load_w(wk)
        hmask_sb = singles.tile([P, HL], BF16)
        nc.sync.dma_start(out=hmask_sb, in_=hmask[:, :])
        wv_sb = load_w(wv)
        wck_sb = load_w(wck)
        wcv_sb = load_w(wcv)
        hsel_sb = singles.tile([HL, P], BF16)
        nc.sync.dma_start(out=hsel_sb, in_=hsel[:, :])
        ident_sb = singles.tile([P, P], BF16)
        nc.sync.dma_start(out=ident_sb, in_=ident[:, :])

        eps2 = singles.tile([HL, 1], F32)
        nc.vector.memset(eps2, EPS)

        # Normed/roped activations in transposed layout.
        qTn = singles.tile([P, N], BF16)
        kTn = singles.tile([P, NK], BF16)
        # V natural layout, per kc per head a 128-wide stationary block:
        # h0: [v(64) | ones(64)], h1: [ones(64) | v(64)]. The ones columns
        # land the softmax denominator on the PSUM rows opposite the o rows.
        v_all = singles.tile([P, KC, 2 * P], BF16)
        nc.gpsimd.memset(v_all, 1.0)
        # Rope tables + rsqrt broadcasts live across both phases.
        cq_sb = singles.tile([P, N], BF16)
        sq_sb = singles.tile([P, N], BF16)
        ckc_sb = singles.tile([P, NK], BF16)
        cks_sb = singles.tile([P, NK], BF16)
        rsb_all = singles.tile([P, 12, 512], BF16)  # q0-3, k0-3, ck0-3

        def rope_chunk(eng, pool, dst, col0, tabC, tabS, tab0, rsb_i):
            """In-place rope over dst[:, col0:col0+512] on engine `eng`."""
            sl = slice(col0, col0 + 512)
            tsl = slice(tab0, tab0 + 512)
            m1 = pool.tile([P, 512], BF16, tag="m1")
            eng.tensor_mul(m1, dst[:, sl], tabC[:, tsl])
            # rotate-half across partitions via SBUF->SBUF DMA (engine-free)
            t1r = pool.tile([P, 512], BF16, tag="t1r")
            for h in range(HL):
                b = h * DH
                nc.sync.dma_start(out=t1r[b:b + 32, :], in_=dst[b + 32:b + 64, sl])
                nc.sync.dma_start(out=t1r[b + 32:b + 64, :], in_=dst[b:b + 32, sl])
            r1 = pool.tile([P, 512], BF16, tag="r1")
            eng.tensor_mul(r1, t1r, tabS[:, tsl])
            s2 = pool.tile([P, 512], BF16, tag="s2")
            eng.tensor_add(s2, m1, r1)
            eng.tensor_mul(dst[:, sl], s2, rsb_all[:, rsb_i, :])

        # ---------------- phase 1: projections + RMSNorm + RoPE ------------
        with tc.tile_pool(name="xy", bufs=3) as xy, \
                tc.tile_pool(name="p1big", bufs=1) as p1big, \
                tc.tile_pool(name="p1work", bufs=4) as p1work, \
                tc.tile_pool(name="fgrope", bufs=2) as fgrope, \
                tc.tile_pool(name="proj_ps", bufs=2, space="PSUM") as proj_ps, \
                tc.tile_pool(name="ssq_ps", bufs=2, space="PSUM") as ssq_ps, \
                tc.tile_pool(name="trps", bufs=2, space="PSUM") as trps, \
                tc.tile_pool(name="rsb_ps", bufs=2, space="PSUM") as rsb_ps:

            vT_sb = p1big.tile([P, NK], BF16)
            ssq_all = p1big.tile([HL, 3 * N], F32)
            rs_all = p1big.tile([HL, 3 * N], BF16)

            def finish_rs(pi):
                """One projection's mean-squares are all in: rsqrt + per-head
                broadcast via selector matmul. Ln/Exp share one ACT table."""
                sl = slice(pi * N, (pi + 1) * N)
                nc.scalar.activation(out=ssq_all[:, sl], in_=ssq_all[:, sl],
                                     func=AF.Ln, bias=eps2)
                nc.scalar.activation(out=rs_all[:, sl], in_=ssq_all[:, sl],
                                     func=AF.Exp, scale=-0.5)
                for t in range(4):
                    rp = rsb_ps.tile([P, 512], F32, tag="rsb")
                    o = pi * N + t * 512
                    nc.tensor.matmul(rp, hsel_sb, rs_all[:, o:o + 512],
                                     start=True, stop=True)
                    nc.scalar.activation(out=rsb_all[:, pi * 4 + t, :], in_=rp,
                                         func=AF.Copy)

            ssq_pend = []
            ssq_done = [0, 0, 0]

            def drain_ssq(keep):
                # mean-square matmuls trail their projection by ~2 slots so
                # the ACT/DVE chains have drained (no in-order PE stall).
                while len(ssq_pend) > keep:
                    qsq, off, pi = ssq_pend.pop(0)
                    sp = ssq_ps.tile([HL, 512], F32, tag="ssq")
                    nc.tensor.matmul(sp, hmask_sb, qsq, start=True, stop=True)
                    nc.scalar.activation(out=ssq_all[:, off:off + 512], in_=sp,
                                         func=AF.Copy)
                    ssq_done[pi] += 1
                    if ssq_done[pi] == 4:
                        finish_rs(pi)

            def proj(w_sb, src, dst_ap, sq_info):
                ps = proj_ps.tile([P, 512], F32, tag="proj")
                for f in range(8):
                    nc.tensor.matmul(ps, w_sb[:, f, :], src[:, f, :],
                                     start=(f == 0), stop=(f == 7))
                if sq_info is None:
                    nc.vector.tensor_copy(dst_ap, ps)  # V path
                else:
                    nc.scalar.activation(out=dst_ap, in_=ps, func=AF.Copy)
                    qsq = p1work.tile([P, 512], BF16, tag="qsq")
                    nc.vector.tensor_mul(qsq, dst_ap, dst_ap)
                    ssq_pend.append((qsq, sq_info[0], sq_info[1]))

            def transpose_group(g):
                trp = trps.tile([P, 4, P], BF16, tag="trp")
                for i in range(4):
                    nc.tensor.transpose(trp[:, i, :],
                                        vT_sb[:, (4 * g + i) * P:(4 * g + i + 1) * P],
                                        ident_sb)
                sl4 = slice(4 * g, 4 * g + 4)
                nc.vector.tensor_copy(v_all[:, sl4, 0:DH], trp[:, :, 0:DH])
                nc.vector.tensor_copy(v_all[:, sl4, 3 * DH:4 * DH],
                                      trp[:, :, DH:2 * DH])

            for t in range(4):
                xt = xy.tile([P, 8, 512], BF16, tag="xy")
                nc.sync.dma_start(out=xt, in_=xT[:, t])
                cs = slice(t * 512, (t + 1) * 512)
                proj(wq_sb, xt, qTn[:, cs], (t * 512, 0))
                proj(wk_sb, xt, kTn[:, cs], (N + t * 512, 1))
                proj(wv_sb, xt, vT_sb[:, cs], None)
                drain_ssq(2)

            # rope tables: loaded after x (startup DMA bandwidth goes to x).
            nc.sync.dma_start(out=cq_sb, in_=cq[:, :])
            nc.sync.dma_start(out=sq_sb, in_=sq[:, :])
            nc.sync.dma_start(out=ckc_sb, in_=ckc[:, :])
            nc.sync.dma_start(out=cks_sb, in_=cks[:, :])

            for t in range(4):
                yt = xy.tile([P, 8, 512], BF16, tag="xy")
                nc.sync.dma_start(out=yt, in_=yT[:, t])
                cs = slice(N + t * 512, N + (t + 1) * 512)
                proj(wck_sb, yt, kTn[:, cs], (2 * N + t * 512, 2))
                proj(wcv_sb, yt, vT_sb[:, cs], None)
                drain_ssq(2)
                transpose_group(t)
            drain_ssq(0)
            for g in range(4, 8):
                transpose_group(g)

            # foreground ropes (DVE): exactly what attention chunk 0 needs
            # first. The rest run on GpSimd/DVE during attention.
            rope_chunk(nc.vector, fgrope, qTn, 0, cq_sb, sq_sb, 0, 0)
            rope_chunk(nc.vector, fgrope, kTn, 0, ckc_sb, cks_sb, 0, 4)
            rope_chunk(nc.vector, fgrope, kTn, 512, ckc_sb, cks_sb, 512, 5)

        # background ropes on the (otherwise idle) GpSimd, consumed
        # progressively by attention chunk 0's kc loop / chunk 1's queries.
        for r in range(2, 8):
            rope_chunk(nc.gpsimd, bgrope, kTn, r * 512, ckc_sb, cks_sb,
                       r * 512, 4 + r)
        rope_chunk(nc.gpsimd, bgrope, qTn, 512, cq_sb, sq_sb, 512, 1)

        # ---------------- phase 2: attention + pipelined A2A + out-proj ----
        with tc.tile_pool(name="pv_ps", bufs=4, space="PSUM") as pv_ps, \
                tc.tile_pool(name="st_ps", bufs=2, space="PSUM") as st_ps, \
                tc.tile_pool(name="zp_ps", bufs=2, space="PSUM") as zp_ps, \
                tc.tile_pool(name="p2work", bufs=6) as p2work, \
                tc.tile_pool(name="p2out", bufs=2) as p2out:

            wo_sb = p2out.tile([P, 8, D], BF16, tag="wo_sb", bufs=1)
            nc.sync.dma_start(out=wo_sb, in_=wo[:, :, :])
            bo_sb = p2out.tile([1, D], BF16, tag="bo_sb", bufs=1)
            nc.sync.dma_start(out=bo_sb, in_=bo[0:1, :])
            bo_b = p2out.tile([P, D], BF16, tag="bo_b", bufs=1)
            nc.gpsimd.partition_broadcast(bo_b[0:P, :], bo_sb[0:1, :],
                                          channels=P)

            def of_load(pair):
                """Collect both chunks of an out-proj pair: one DMA each."""
                of = p2out.tile([P, NCORES, P], BF16, tag="of")
                for half in range(2):
                    c = pair * 2 + half
                    nc.sync.dma_start(
                        out=of[:, :, half * CPC:(half + 1) * CPC],
                        in_=a2a_out[c][:, :, :].transpose([1, 0, 2]))
                return of

            def outproj(pair, of):
                for nn in range(2):
                    zp = zp_ps.tile([P, 512], F32, tag="zp")
                    for j in range(NCORES):
                        nc.tensor.matmul(zp, of[:, j, :],
                                         wo_sb[:, j, nn * 512:(nn + 1) * 512],
                                         start=(j == 0), stop=(j == NCORES - 1))
                    zs = p2out.tile([P, 512], F32, tag="zs")
                    nc.vector.tensor_add(zs, zp,
                                         bo_b[:, nn * 512:(nn + 1) * 512])
                    nc.sync.dma_start(out=out_ext[pair * P:(pair + 1) * P,
                                                  nn * 512:(nn + 1) * 512],
                                      in_=zs)

            of0 = None
            for c in range(NCH):
                cs = slice(c * CQ, (c + 1) * CQ)
                pv = [pv_ps.tile([P, CQ], F32, tag="pv", name=f"pv{c}_{h}")
                      for h in range(HL)]

                def emit_pv(kc, es):
                    for h in range(HL):
                        nc.tensor.matmul(
                            pv[h], v_all[:, kc, h * P:(h + 1) * P], es[h],
                            start=(kc == 0), stop=(kc == KC - 1))

                es_prev = None
                for kc in range(KC):
                    sts = [st_ps.tile([P, CQ], F32, tag="st",
                                      name=f"st{c}_{kc}_{h}")
                           for h in range(HL)]
                    for h in range(HL):
                        hs = slice(h * DH, (h + 1) * DH)
                        nc.tensor.matmul(sts[h], kTn[hs, kc * P:(kc + 1) * P],
                                         qTn[hs, cs], start=True, stop=True)
                    # previous kc's PV fills the PE while this kc's exps run
                    if es_prev is not None:
                        emit_pv(kc - 1, es_prev)
                    es = []
                    for h in range(HL):
                        e = p2work.tile([P, CQ], BF16, tag="es", bufs=6)
                        if h == 0:
                            nc.scalar.activation(out=e, in_=sts[h],
                                                 func=AF.Exp, scale=SCALE)
                        else:
                            # Schraudolph bf16 exp on the DVE
                            nc.vector.tensor_scalar(
                                out=e.bitcast(I16), in0=sts[h],
                                scalar1=EXA, scalar2=EXB,
                                op0=mybir.AluOpType.mult,
                                op1=mybir.AluOpType.add)
                        es.append(e)
                    es_prev = es
                emit_pv(KC - 1, es_prev)

                # normalize. denominators sit on the PSUM rows opposite the
                # o rows (64 copies); recip = Ln + Exp(-x) on those rows
                # (same ACT table as softmax exp), then a partition-shift
                # DMA aligns them with the o rows.
                oT = p2work.tile([P, CQ], BF16, tag="oT", bufs=2)
                lnd = p2work.tile([P, CQ], F32, tag="lnd", bufs=2)
                rdc = p2work.tile([P, CQ], BF16, tag="rdc", bufs=2)
                rdb = p2work.tile([P, CQ], BF16, tag="rdb", bufs=2)
                nc.scalar.activation(out=lnd[DH:P, :], in_=pv[0][DH:P, :],
                                     func=AF.Ln)
                nc.scalar.activation(out=rdc[DH:P, :], in_=lnd[DH:P, :],
                                     func=AF.Exp, scale=-1.0)
                nc.sync.dma_start(out=rdb[0:DH, :], in_=rdc[DH:P, :])
                nc.vector.tensor_mul(oT[0:DH, :], pv[0][0:DH, :], rdb[0:DH, :])
                nc.scalar.activation(out=lnd[0:DH, :], in_=pv[1][0:DH, :],
                                     func=AF.Ln)
                nc.scalar.activation(out=rdc[0:DH, :], in_=lnd[0:DH, :],
                                     func=AF.Exp, scale=-1.0)
                nc.sync.dma_start(out=rdb[DH:P, :], in_=rdc[0:DH, :])
                nc.vector.tensor_mul(oT[DH:P, :], pv[1][DH:P, :], rdb[DH:P, :])

                # A2A for this chunk: dest core j gets positions j*64..+64.
                nc.sync.dma_start(
                    out=a2a_in[c][:, :, :].transpose([1, 0, 2]), in_=oT[:, :])
                nc.gpsimd.collective_compute(
                    "AllToAll", mybir.AluOpType.bypass,
                    replica_groups=[list(range(NCORES))],
                    ins=[a2a_in[c][:, :, :]],
                    outs=[a2a_out[c][:, :, :]],
                )

                # boundary fillers on the DVE (cheap, between chunk exps)
                if c == 0:
                    rope_chunk(nc.vector, bgrope, qTn, 2 * 512, cq_sb, sq_sb,
                               2 * 512, 2)
                elif c == 1:
                    of0 = of_load(0)
                    rope_chunk(nc.vector, bgrope, qTn, 3 * 512, cq_sb, sq_sb,
                               3 * 512, 3)

            # out-projections after chunk 3: pair 0's matmuls execute inside
            # chunk 3's A2A flight window; only pair 1's tail is exposed.
            of1 = of_load(1)
            outproj(0, of0)
            outproj(1, of1)
    return nc


def _bf16(a):
    return np.ascontiguousarray(a).astype(ml_dtypes.bfloat16)


def _rope_tables(npos, pos0, g_first, g_second, n_first):
    """Tables [128, npos] for transposed-layout rope with g folded in.

    Row j (within a head, duplicated for 2 local heads):
      out[j] = t[j]*C[j] + t[sigma(j)]*S[j]
      j <  32: C[j]=g[j]*cos[n,j],     S[j]=-g[j+32]*sin[n,j]
      j >= 32: C[j]=g[j]*cos[n,j-32],  S[j]=+g[j-32]*sin[n,j-32]
    g switches from g_first to g_second at position n_first.
    """
    inv = 1.0 / (ROPE_BASE ** (np.arange(0, DH, 2, dtype=np.float64) / DH))
    pos = np.arange(pos0, pos0 + npos, dtype=np.float64)
    ang = pos[:, None] * inv[None, :]          # [npos, 32]
    cos = np.cos(ang).T                         # [32, npos]
    sin = np.sin(ang).T
    C = np.zeros((DH, npos), np.float64)
    S = np.zeros((DH, npos), np.float64)
    g = np.zeros((DH, npos), np.float64)
    g[:, :n_first] = np.asarray(g_first, np.float64)[:, None]
    if n_first < npos:
        g[:, n_first:] = np.asarray(g_second, np.float64)[:, None]
    C[:32] = cos
    C[32:] = cos
    C *= g
    S[:32] = -sin
    S[32:] = sin
    Srot = np.concatenate([g[32:], g[:32]], axis=0)  # g[sigma(j)]
    S *= Srot
    C2 = np.concatenate([C, C], axis=0)  # duplicate for 2 local heads
    S2 = np.concatenate([S, S], axis=0)
    return _bf16(C2), _bf16(S2)


_NC_CACHE = None


def kernel(x, y, W_qkv, W_ckv, W_out, b_out, g_q, g_k, g_ck, n_heads):
    global LAST_RESULT, _NC_CACHE
    x = np.asarray(x, np.float32)
    y = np.asarray(y, np.float32)
    W_qkv = np.asarray(W_qkv, np.float32)
    W_ckv = np.asarray(W_ckv, np.float32)
    W_out = np.asarray(W_out, np.float32)
    b_out = np.asarray(b_out, np.float32)

    def _prearr_x(a):
        # a [2048 pos, 1024 feat] -> [p, chunk, f, 512]:
        # element (f*128+p, c*512+ns) lands at [p, c, f, ns]
        return _bf16(a.T.reshape(8, P, 4, 512).transpose(1, 2, 0, 3))

    xT = _prearr_x(x[0])
    yT = _prearr_x(y[0])
    Wq, Wk, Wv = (W_qkv[:, i * D:(i + 1) * D] for i in range(3))
    Wck, Wcv = (W_ckv[:, i * D:(i + 1) * D] for i in range(2))

    def _prearr(w):
        # [1024, C] row f*128+p -> [p, f, c]: contiguous per-partition DMAs
        return _bf16(w.reshape(8, P, -1).transpose(1, 0, 2))

    woh = _prearr(W_out)
    boh = _bf16(b_out[None, :])

    cqh, sqh = _rope_tables(N, 0, g_q, g_q, N)
    ckch, cksh = _rope_tables(NK, 0, g_k, g_ck, N)
    hm = np.zeros((P, HL), np.float32)
    for h in range(HL):
        hm[h * DH:(h + 1) * DH, h] = 1.0 / DH
    hmh = _bf16(hm)
    hs = np.zeros((HL, P), np.float32)
    for h in range(HL):
        hs[h, h * DH:(h + 1) * DH] = 1.0
    hsh = _bf16(hs)
    idh = _bf16(np.eye(P, dtype=np.float32))

    in_maps = []
    for c in range(NCORES):
        sl = slice(c * DL, (c + 1) * DL)
        in_maps.append({
            "xT": xT, "yT": yT,
            "wq": _prearr(Wq[:, sl]), "wk": _prearr(Wk[:, sl]),
            "wv": _prearr(Wv[:, sl]), "wck": _prearr(Wck[:, sl]),
            "wcv": _prearr(Wcv[:, sl]),
            "wo": woh, "bo": boh,
            "cq": cqh, "sq": sqh, "ckc": ckch, "cks": cksh,
            "hmask": hmh, "hsel": hsh, "ident": idh,
        })

    if _NC_CACHE is None:
        _NC_CACHE = build_nc()
        if not _NC_CACHE.is_finalized():
            _NC_CACHE.finalize()
    nc = _NC_CACHE

    res = run_bass_kernel_spmd(
        nc, in_maps, core_ids=list(range(NCORES)),
        trace=bool(os.environ.get("BASS_TRACE")),
    )
    LAST_RESULT = res
    # out_ext rows on core j: pair*128 + half*64 + cc
    #   <-> global position (pair*2 + half)*512 + j*64 + cc
    out = np.empty((N, D), np.float32)
    for j in range(NCORES):
        o = np.asarray(res.results[j]["out"], np.float32)
        for ch in range(NCH):
            out[ch * CQ + j * CPC:ch * CQ + (j + 1) * CPC] = \
                o[ch * CPC:(ch + 1) * CPC]
    return out[None, :, :]
